# revision 32
# baseline (speedup 1.0000x reference)
"""Pipelined GEMM kernel for Trainium2, 8 NeuronCores.

Computes C = A @ B + ws*(ws+1)/2 with A:(8192,256) B:(256,8192) fp32.

Sharding: 2x4 grid over (M, N). Core (mi, ni) computes the (4096, 2048)
output block from A rows [mi] (staged K-major) and B columns [ni].

Precision strategy (rel-err budget 2e-2, we spend 1.524e-2, measured
deterministic on the seeded inputs):
  - inputs host-cast to fp8 e4m3 (1.5MB/core loads); matmuls run in
    DoubleRow perf mode - 2 fp8 weights per PE cell consume both
    128-row k-subtiles in ONE pass at 0.5 cycles/row, halving PE time
    vs bf16. PSUM accumulation stays fp32. (in_dtype="bf16" fallback:
    rel err 9.76e-4, ~25% slower.)
  - output written fp16 (+const fused into eviction), host-upcast.

Measured (HW, paired repeat-slope): 52-59us/exec vs 120us baseline
(run-to-run thermal/P0 drift; best official print 51891ns).
Progression: 120 (fp32 I/O baseline) -> 87 (bf16 in / fp16 out) -> 77
(2-bank psum tiles, single-reader DVE/ACT evictions) -> 73 (unroll-2
software pipeline) -> 59 (fp8 DoubleRow). Now bounded by eviction
engine time + HBM stores (~17.5MB/core at ~358GB/s), no longer PE.

Structure per body (Tile framework):
  - repeat-loop timing: prologue body (with 5 HAM-warmup matmuls on a
    memset tile) outside the HW loop; the loop is unrolled x2 over
    double-buffered input tile sets, each body prefetching the OTHER
    set's inputs - no reload gate between bodies.
  - inputs live in 3D [128, 2, *] fp8 tiles (k-subtile middle dim, the
    [Ki, Ko=2, dim] AP DoubleRow wants); the cold prologue loads
    512-col pieces first (early m-tile 0 start), in-loop prefetch uses
    4 whole-k-tile DMAs (fewer descriptors on a near-saturated SDMA
    budget); k-subtile k rides the sync/scalar HWDGE ring k.
  - 32 m-tiles x 4 DoubleRow matmuls into two [128, 1024] fp32 psum
    tiles (4-deep pool = all 8 banks), single eviction reader each,
    DVE tensor_scalar_add / ACT activation(Copy, bias) alternating by
    (m+jj) parity (measured 1.26us / 1.15us per [128,1024], +7%
    concurrent).
  - stores: 4 m-tiles = one 2MB fp16 DMA, rings alternating; last
    group split 1MB/0.5MB/2x0.25MB to shorten the drain tail.

Tried and rejected (HW-measured): ldweights dedup (bacc already splits
matmuls into LDW+MM pairs; the PE reorder window hides them - and on
the fp8 DoubleRow stream removing the duplicate LDWs HANGS the device:
never enable ldw_dedup with in_dtype="fp8"); k-outer same-weight
matmul orders; two-reader [128,2048] psum tiles; fp8 WITHOUT DoubleRow
(no speedup - 1 elem/cell/cycle regardless of width); fp8 output store
(adds ~0.9e-2 err: total 1.78e-2 leaves only 11% gate margin).
"""

import contextlib

import numpy as np
import ml_dtypes

import concourse.mybir as mybir
import concourse.tile as tile
from concourse import bacc
from concourse.bass_utils import run_bass_kernel_spmd

M, K, N = 8192, 256, 8192
NCORES = 8
RM, RN = 2, 4  # core grid over (M, N)
MS = M // RM  # 4096 rows of C per core
NS = N // RN  # 2048 cols of C per core
P = 128
MT = MS // P  # 32 m-tiles
KT = K // P  # 2 k-tiles
NCHUNK = 512  # one fp32 PSUM bank / max matmul free dim
LP = 1024  # load piece width (cols)
GROUP = 4  # m-tiles per store DMA (2MB fp16)

F32 = mybir.dt.float32
F16 = mybir.dt.float16
BF16 = mybir.dt.bfloat16
F8 = mybir.dt.float8e4
BF16_NP = np.dtype(ml_dtypes.bfloat16)
F8_NP = np.dtype(ml_dtypes.float8_e4m3)


def build_program(const_add: float, repeat: int = 1, loop_opts: dict | None = None,
                  kt_used: int = KT, do_evict: bool = True, do_store: bool = True,
                  nwarm: int = 5, opool_bufs: int = 4, unroll2: bool = True,
                  ldw_dedup: bool = False, in_dtype: str = "fp8"):
    """repeat>1 wraps the body in a HW loop - used only by the timing
    harness (slope between two repeat counts cancels the ~130ms axon
    dispatch overhead). With unroll2, the loop is unrolled by two over
    double-buffered input tile sets: each body prefetches the OTHER
    set's inputs, so successive bodies pipeline with no reload gate,
    and the warmup-carrying prologue stays outside the loop (the slope
    then measures the lean steady-state body)."""
    nc = bacc.Bacc("TRN2", target_bir_lowering=False, debug=False)
    idt = F8 if in_dtype == "fp8" else BF16
    at = nc.dram_tensor("at", [K, MS], idt, kind="ExternalInput")
    b = nc.dram_tensor("b", [K, NS], idt, kind="ExternalInput")
    c = nc.dram_tensor("c", [MS, NS], F16, kind="ExternalOutput")

    with tile.TileContext(nc) as tc:
        with (
            tc.tile_pool(name="bpool", bufs=1) as bpool,
            tc.tile_pool(name="atpool", bufs=1) as atpool,
            tc.tile_pool(name="psum", bufs=4, space="PSUM") as psum_pool,
            tc.tile_pool(name="opool", bufs=opool_bufs) as opool,
            tc.tile_pool(name="scratch", bufs=2) as scratch_pool,
        ):
            nsets = 2 if (unroll2 and repeat > 1) else 1
            if in_dtype == "fp8":
                # one 3D [P, KT, *] tile per set: the k-subtile axis is
                # the middle dim, as DoubleRow's [Ki, Ko=2, dim] AP wants
                at_sets = [
                    atpool.tile([P, KT, MS], F8, name=f"at8s{s}", tag=f"at8s{s}")
                    for s in range(nsets)
                ]
                b_sets = [
                    bpool.tile([P, KT, NS], F8, name=f"b8s{s}", tag=f"b8s{s}")
                    for s in range(nsets)
                ]
            else:
                at_sets = [
                    [atpool.tile([P, MS], BF16, name=f"at{k}s{s}", tag=f"at{k}s{s}")
                     for k in range(KT)]
                    for s in range(nsets)
                ]
                b_sets = [
                    [bpool.tile([P, NS], BF16, name=f"b{k}s{s}", tag=f"b{k}s{s}")
                     for k in range(KT)]
                    for s in range(nsets)
                ]

            def ring(k):
                return nc.sync if k == 0 else nc.scalar

            def emit_loads(at_sb, b_sb, fine=True):
                # k-tile k rides ring k. fine=True: 512-col pieces
                # first so m-tile 0 can start after ~0.5MB (cold-start
                # prologue). fine=False: whole k-tiles in 4 big DMAs -
                # in-loop prefetch has a full body of slack, and big
                # transfers waste less SDMA time on descriptors.
                def bsl(k, c0, c1):
                    return b_sb[:, k, c0:c1] if in_dtype == "fp8" else b_sb[k][:, c0:c1]

                def asl(k, c0, c1):
                    return (at_sb[:, k, c0:c1] if in_dtype == "fp8"
                            else at_sb[k][:, c0:c1])

                if not fine:
                    for k in range(KT):
                        ring(k).dma_start(bsl(k, 0, NS), b[k * P : (k + 1) * P, :])
                    for k in range(KT):
                        ring(k).dma_start(asl(k, 0, MS), at[k * P : (k + 1) * P, :])
                    return

                FL = NCHUNK
                for k in range(KT):
                    ring(k).dma_start(bsl(k, 0, FL), b[k * P : (k + 1) * P, 0:FL])
                for k in range(KT):
                    ring(k).dma_start(asl(k, 0, FL), at[k * P : (k + 1) * P, 0:FL])
                for k in range(KT):
                    ring(k).dma_start(
                        bsl(k, FL, 2 * FL), b[k * P : (k + 1) * P, FL : 2 * FL])
                for k in range(KT):
                    ring(k).dma_start(
                        asl(k, FL, 2 * FL), at[k * P : (k + 1) * P, FL : 2 * FL])
                for k in range(KT):
                    ring(k).dma_start(bsl(k, LP, NS), b[k * P : (k + 1) * P, LP:NS])
                for p0 in range(LP, MS, LP):
                    for k in range(KT):
                        ring(k).dma_start(
                            asl(k, p0, p0 + LP),
                            at[k * P : (k + 1) * P, p0 : p0 + LP])

            def emit_compute(at_sb, b_sb, warm):
                # Dummy matmuls on a memset tile warm the PE clock gate
                # (HAM un-throttles after ~3.4us of activity) while the
                # first loads are in flight.
                if warm and nwarm:
                    dw = scratch_pool.tile([P, NCHUNK], BF16, name="dw", tag="dw")
                    nc.vector.memset(dw[:], 0.0)
                    ps = psum_pool.tile([P, 2 * NCHUNK], F32, name="ps")
                    for _ in range(nwarm):
                        nc.tensor.matmul(ps[:, 0:NCHUNK], dw[:, 0:P], dw[:],
                                         start=True, stop=True)

                NG = MT // GROUP
                half = NS // 2
                for g in range(NG):
                    ot = opool.tile([P, GROUP * NS], F16, name="ot")
                    for mh in range(GROUP):
                        m = g * GROUP + mh
                        dst = ot[:, mh * NS : (mh + 1) * NS]
                        # 2-bank psum tiles: each bank pair completes
                        # after 4 matmuls and its eviction (DVE / ACT
                        # alternating, single reader per tile) overlaps
                        # the rest.
                        for jj in range(2):
                            ps = psum_pool.tile([P, 2 * NCHUNK], F32, name="ps")
                            for j2 in range(2):
                                j = jj * 2 + j2
                                if in_dtype == "fp8":
                                    # DoubleRow: both k-subtiles in one
                                    # pass (2 fp8 weights per PE cell)
                                    nc.tensor.matmul(
                                        ps[:, j2 * NCHUNK : (j2 + 1) * NCHUNK],
                                        at_sb[:, 0:KT, m * P : (m + 1) * P],
                                        b_sb[:, 0:KT, j * NCHUNK : (j + 1) * NCHUNK],
                                        start=True,
                                        stop=True,
                                        perf_mode=mybir.MatmulPerfMode.DoubleRow,
                                    )
                                    continue
                                for k in range(kt_used):
                                    nc.tensor.matmul(
                                        ps[:, j2 * NCHUNK : (j2 + 1) * NCHUNK],
                                        at_sb[k][:, m * P : (m + 1) * P],
                                        b_sb[k][:, j * NCHUNK : (j + 1) * NCHUNK],
                                        start=(k == 0),
                                        stop=(k == kt_used - 1),
                                    )
                            if do_evict:
                                d = dst[:, jj * 2 * NCHUNK : (jj + 1) * 2 * NCHUNK]
                                if (m + jj) % 2 == 0:
                                    nc.vector.tensor_scalar_add(d, ps[:], const_add)
                                else:
                                    nc.scalar.activation(
                                        d, ps[:],
                                        mybir.ActivationFunctionType.Copy,
                                        bias=const_add,
                                    )
                    base = g * GROUP * P
                    if not do_store:
                        pass
                    elif g < NG - 1:
                        eng = nc.sync if g % 2 == 0 else nc.scalar
                        eng.dma_start(
                            c[base : base + GROUP * P, :].rearrange(
                                "(h p) n -> p h n", p=P),
                            ot[:],
                        )
                    else:
                        # tail split: shorten the serial drain after the
                        # last eviction (1MB + 0.5MB + 2 x 0.25MB).
                        nc.sync.dma_start(
                            c[base : base + 2 * P, :].rearrange(
                                "(h p) n -> p h n", p=P),
                            ot[:, 0 : 2 * NS],
                        )
                        nc.scalar.dma_start(
                            c[base + 2 * P : base + 3 * P, :], ot[:, 2 * NS : 3 * NS]
                        )
                        nc.sync.dma_start(
                            c[base + 3 * P : base + 4 * P, 0:half],
                            ot[:, 3 * NS : 3 * NS + half],
                        )
                        nc.scalar.dma_start(
                            c[base + 3 * P : base + 4 * P, half:NS],
                            ot[:, 3 * NS + half : 4 * NS],
                        )

            if repeat == 1:
                emit_loads(at_sets[0], b_sets[0])
                emit_compute(at_sets[0], b_sets[0], warm=True)
            elif not unroll2:
                with tc.For_i(0, repeat, 1, **(loop_opts or {})):
                    emit_loads(at_sets[0], b_sets[0])
                    emit_compute(at_sets[0], b_sets[0], warm=True)
            else:
                # prologue: load+run set 0, prefetch set 1 behind it
                emit_loads(at_sets[0], b_sets[0])
                emit_loads(at_sets[1], b_sets[1], fine=False)
                emit_compute(at_sets[0], b_sets[0], warm=True)
                n2, rem = divmod(repeat - 1, 2)
                if n2:
                    with tc.For_i(0, n2, 1, **(loop_opts or {})):
                        # each body prefetches the other set's inputs
                        emit_loads(at_sets[0], b_sets[0], fine=False)
                        emit_compute(at_sets[1], b_sets[1], warm=False)
                        emit_loads(at_sets[1], b_sets[1], fine=False)
                        emit_compute(at_sets[0], b_sets[0], warm=False)
                if rem:
                    emit_compute(at_sets[1], b_sets[1], warm=False)

    nc.compile()
    if ldw_dedup:
        _dedup_ldweights(nc)
    return nc


def _dedup_ldweights(nc):
    """Post-compile: drop InstLdweights whose weights AP equals the
    currently-loaded one. bacc splits every matmul into an explicit
    InstLdweights + InstMatmult(ldweights=False) pair, so with a
    same-weight-run matmul order most loads are redundant. Only loads
    with no waits/updates are dropped; tracking resets at block
    boundaries and drains."""
    f = nc.m.functions[0]
    for bb in f.blocks:
        insts = bb.instructions
        out = []
        loaded = None
        for inst in insts:
            nm = type(inst).__name__
            if str(getattr(inst, "engine", "")) != "EngineType.PE":
                out.append(inst)
                continue
            if nm == "InstLdweights":
                sig = str(inst.ins[0])
                if (sig == loaded and not getattr(inst, "waits", None)
                        and not getattr(inst, "updates", None)):
                    continue  # redundant reload
                loaded = sig
                out.append(inst)
            elif nm == "InstMatmult":
                out.append(inst)
            else:
                loaded = None  # drain / branch / unknown: be conservative
                out.append(inst)
        if len(out) != len(insts):
            bb.instructions = out


_CACHE = {}


def _get_program(const_add: float):
    key = const_add
    if key not in _CACHE:
        _CACHE[key] = build_program(const_add)
    return _CACHE[key]


def make_in_maps(A, B, in_dtype: str = "fp8"):
    """2x4 (M, N) grid; shards host-cast to fp8/bf16, A staged K-major."""
    np_dt = F8_NP if in_dtype == "fp8" else BF16_NP
    A16 = np.asarray(A, dtype=np.float32).astype(np_dt)
    B16 = np.asarray(B, dtype=np.float32).astype(np_dt)
    maps = []
    for i in range(NCORES):
        mi, ni = divmod(i, RN)
        maps.append({
            "at": np.ascontiguousarray(A16[mi * MS : (mi + 1) * MS].T),
            "b": np.ascontiguousarray(B16[:, ni * NS : (ni + 1) * NS]),
        })
    return maps


def assemble(results):
    rows = []
    for mi in range(RM):
        rows.append(np.concatenate(
            [results[mi * RN + ni]["c"] for ni in range(RN)], axis=1))
    return np.concatenate(rows, axis=0).astype(np.float32)


def run(A, B, world_size, trace=False, **spmd_kwargs):
    ws = int(world_size)
    const_add = float(ws * (ws + 1) / 2)
    A = np.asarray(A)
    B = np.asarray(B)
    assert A.shape == (M, K) and B.shape == (K, N)

    nc = _get_program(const_add)
    res = run_bass_kernel_spmd(
        nc, make_in_maps(A, B), list(range(NCORES)), trace=trace, **spmd_kwargs
    )
    return assemble(res.results), res


def kernel(A, B, world_size, **_unused):
    out, _ = run(A, B, world_size, trace=False)
    return out


# revision 34
# speedup vs baseline: 1.0618x; 1.0618x over previous
"""Pipelined GEMM kernel for Trainium2, 8 NeuronCores.

Computes C = A @ B + ws*(ws+1)/2 with A:(8192,256) B:(256,8192) fp32.

Sharding: 2x4 grid over (M, N). Core (mi, ni) computes the (4096, 2048)
output block from A rows [mi] (staged K-major) and B columns [ni].

Precision strategy (rel-err budget 2e-2, we spend 1.524e-2, measured
deterministic on the seeded inputs):
  - inputs host-cast to fp8 e4m3 (1.5MB/core loads); matmuls run in
    DoubleRow perf mode - 2 fp8 weights per PE cell consume both
    128-row k-subtiles in ONE pass at 0.5 cycles/row, halving PE time
    vs bf16. PSUM accumulation stays fp32. (in_dtype="bf16" fallback:
    rel err 9.76e-4, ~25% slower.)
  - output written fp16 (+const fused into eviction), host-upcast.

Measured (HW, paired repeat-slope): 52-59us/exec vs 120us baseline
(run-to-run thermal/P0 drift; best official print 51891ns).
Progression: 120 (fp32 I/O baseline) -> 87 (bf16 in / fp16 out) -> 77
(2-bank psum tiles, single-reader DVE/ACT evictions) -> 73 (unroll-2
software pipeline) -> 59 (fp8 DoubleRow). Now bounded by eviction
engine time + HBM stores (~17.5MB/core at ~358GB/s), no longer PE.

Structure per body (Tile framework):
  - repeat-loop timing: prologue body (with 5 HAM-warmup matmuls on a
    memset tile) outside the HW loop; the loop is unrolled x2 over
    double-buffered input tile sets, each body prefetching the OTHER
    set's inputs - no reload gate between bodies.
  - inputs live in 3D [128, 2, *] fp8 tiles (k-subtile middle dim, the
    [Ki, Ko=2, dim] AP DoubleRow wants); the cold prologue loads
    512-col pieces first (early m-tile 0 start), in-loop prefetch uses
    4 whole-k-tile DMAs (fewer descriptors on a near-saturated SDMA
    budget); k-subtile k rides the sync/scalar HWDGE ring k.
  - 32 m-tiles x 4 DoubleRow matmuls into two [128, 1024] fp32 psum
    tiles (4-deep pool = all 8 banks), single eviction reader each,
    DVE tensor_scalar_add / ACT activation(Copy, bias) alternating by
    (m+jj) parity (measured 1.26us / 1.15us per [128,1024], +7%
    concurrent).
  - stores: 8 m-tiles = one 4MB fp16 DMA (93%+ SDMA efficiency; the
    SDMA timeline is the binding resource), rings alternating; last
    group split 3MB/0.5MB/2x0.25MB to shorten the drain tail. HW A/B:
    sgroup=8 50.4us vs sgroup=4 52.4-53.9us.

Tried and rejected (HW-measured): ldweights dedup (bacc already splits
matmuls into LDW+MM pairs; the PE reorder window hides them - and on
the fp8 DoubleRow stream removing the duplicate LDWs HANGS the device:
never enable ldw_dedup with in_dtype="fp8"); k-outer same-weight
matmul orders; two-reader [128,2048] psum tiles; fp8 WITHOUT DoubleRow
(no speedup - 1 elem/cell/cycle regardless of width); fp8 output store
(adds ~0.9e-2 err: total 1.78e-2 leaves only 11% gate margin).
"""

import contextlib

import numpy as np
import ml_dtypes

import concourse.mybir as mybir
import concourse.tile as tile
from concourse import bacc
from concourse.bass_utils import run_bass_kernel_spmd

M, K, N = 8192, 256, 8192
NCORES = 8
RM, RN = 2, 4  # core grid over (M, N)
MS = M // RM  # 4096 rows of C per core
NS = N // RN  # 2048 cols of C per core
P = 128
MT = MS // P  # 32 m-tiles
KT = K // P  # 2 k-tiles
NCHUNK = 512  # one fp32 PSUM bank / max matmul free dim
LP = 1024  # load piece width (cols)
GROUP = 4  # legacy default; production uses sgroup=8 (4MB fp16 stores)

F32 = mybir.dt.float32
F16 = mybir.dt.float16
BF16 = mybir.dt.bfloat16
F8 = mybir.dt.float8e4
BF16_NP = np.dtype(ml_dtypes.bfloat16)
F8_NP = np.dtype(ml_dtypes.float8_e4m3)


def build_program(const_add: float, repeat: int = 1, loop_opts: dict | None = None,
                  kt_used: int = KT, do_evict: bool = True, do_store: bool = True,
                  nwarm: int = 5, opool_bufs: int = 3, unroll2: bool = True,
                  ldw_dedup: bool = False, in_dtype: str = "fp8",
                  sgroup: int = 8):
    """repeat>1 wraps the body in a HW loop - used only by the timing
    harness (slope between two repeat counts cancels the ~130ms axon
    dispatch overhead). With unroll2, the loop is unrolled by two over
    double-buffered input tile sets: each body prefetches the OTHER
    set's inputs, so successive bodies pipeline with no reload gate,
    and the warmup-carrying prologue stays outside the loop (the slope
    then measures the lean steady-state body)."""
    nc = bacc.Bacc("TRN2", target_bir_lowering=False, debug=False)
    idt = F8 if in_dtype == "fp8" else BF16
    at = nc.dram_tensor("at", [K, MS], idt, kind="ExternalInput")
    b = nc.dram_tensor("b", [K, NS], idt, kind="ExternalInput")
    c = nc.dram_tensor("c", [MS, NS], F16, kind="ExternalOutput")

    with tile.TileContext(nc) as tc:
        with (
            tc.tile_pool(name="bpool", bufs=1) as bpool,
            tc.tile_pool(name="atpool", bufs=1) as atpool,
            tc.tile_pool(name="psum", bufs=4, space="PSUM") as psum_pool,
            tc.tile_pool(name="opool", bufs=opool_bufs) as opool,
            tc.tile_pool(name="scratch", bufs=2) as scratch_pool,
        ):
            nsets = 2 if (unroll2 and repeat > 1) else 1
            if in_dtype == "fp8":
                # one 3D [P, KT, *] tile per set: the k-subtile axis is
                # the middle dim, as DoubleRow's [Ki, Ko=2, dim] AP wants
                at_sets = [
                    atpool.tile([P, KT, MS], F8, name=f"at8s{s}", tag=f"at8s{s}")
                    for s in range(nsets)
                ]
                b_sets = [
                    bpool.tile([P, KT, NS], F8, name=f"b8s{s}", tag=f"b8s{s}")
                    for s in range(nsets)
                ]
            else:
                at_sets = [
                    [atpool.tile([P, MS], BF16, name=f"at{k}s{s}", tag=f"at{k}s{s}")
                     for k in range(KT)]
                    for s in range(nsets)
                ]
                b_sets = [
                    [bpool.tile([P, NS], BF16, name=f"b{k}s{s}", tag=f"b{k}s{s}")
                     for k in range(KT)]
                    for s in range(nsets)
                ]

            def ring(k):
                return nc.sync if k == 0 else nc.scalar

            def emit_loads(at_sb, b_sb, fine=True):
                # k-tile k rides ring k. fine=True: 512-col pieces
                # first so m-tile 0 can start after ~0.5MB (cold-start
                # prologue). fine=False: whole k-tiles in 4 big DMAs -
                # in-loop prefetch has a full body of slack, and big
                # transfers waste less SDMA time on descriptors.
                def bsl(k, c0, c1):
                    return b_sb[:, k, c0:c1] if in_dtype == "fp8" else b_sb[k][:, c0:c1]

                def asl(k, c0, c1):
                    return (at_sb[:, k, c0:c1] if in_dtype == "fp8"
                            else at_sb[k][:, c0:c1])

                if not fine:
                    for k in range(KT):
                        ring(k).dma_start(bsl(k, 0, NS), b[k * P : (k + 1) * P, :])
                    for k in range(KT):
                        ring(k).dma_start(asl(k, 0, MS), at[k * P : (k + 1) * P, :])
                    return

                FL = NCHUNK
                for k in range(KT):
                    ring(k).dma_start(bsl(k, 0, FL), b[k * P : (k + 1) * P, 0:FL])
                for k in range(KT):
                    ring(k).dma_start(asl(k, 0, FL), at[k * P : (k + 1) * P, 0:FL])
                for k in range(KT):
                    ring(k).dma_start(
                        bsl(k, FL, 2 * FL), b[k * P : (k + 1) * P, FL : 2 * FL])
                for k in range(KT):
                    ring(k).dma_start(
                        asl(k, FL, 2 * FL), at[k * P : (k + 1) * P, FL : 2 * FL])
                for k in range(KT):
                    ring(k).dma_start(bsl(k, LP, NS), b[k * P : (k + 1) * P, LP:NS])
                for p0 in range(LP, MS, LP):
                    for k in range(KT):
                        ring(k).dma_start(
                            asl(k, p0, p0 + LP),
                            at[k * P : (k + 1) * P, p0 : p0 + LP])

            def emit_compute(at_sb, b_sb, warm):
                # Dummy matmuls on a memset tile warm the PE clock gate
                # (HAM un-throttles after ~3.4us of activity) while the
                # first loads are in flight.
                if warm and nwarm:
                    dw = scratch_pool.tile([P, NCHUNK], BF16, name="dw", tag="dw")
                    nc.vector.memset(dw[:], 0.0)
                    ps = psum_pool.tile([P, 2 * NCHUNK], F32, name="ps")
                    for _ in range(nwarm):
                        nc.tensor.matmul(ps[:, 0:NCHUNK], dw[:, 0:P], dw[:],
                                         start=True, stop=True)

                NG = MT // sgroup
                half = NS // 2
                for g in range(NG):
                    ot = opool.tile([P, sgroup * NS], F16, name="ot")
                    for mh in range(sgroup):
                        m = g * sgroup + mh
                        dst = ot[:, mh * NS : (mh + 1) * NS]
                        # 2-bank psum tiles: each bank pair completes
                        # after 4 matmuls and its eviction (DVE / ACT
                        # alternating, single reader per tile) overlaps
                        # the rest.
                        for jj in range(2):
                            ps = psum_pool.tile([P, 2 * NCHUNK], F32, name="ps")
                            for j2 in range(2):
                                j = jj * 2 + j2
                                if in_dtype == "fp8":
                                    # DoubleRow: both k-subtiles in one
                                    # pass (2 fp8 weights per PE cell)
                                    nc.tensor.matmul(
                                        ps[:, j2 * NCHUNK : (j2 + 1) * NCHUNK],
                                        at_sb[:, 0:KT, m * P : (m + 1) * P],
                                        b_sb[:, 0:KT, j * NCHUNK : (j + 1) * NCHUNK],
                                        start=True,
                                        stop=True,
                                        perf_mode=mybir.MatmulPerfMode.DoubleRow,
                                    )
                                    continue
                                for k in range(kt_used):
                                    nc.tensor.matmul(
                                        ps[:, j2 * NCHUNK : (j2 + 1) * NCHUNK],
                                        at_sb[k][:, m * P : (m + 1) * P],
                                        b_sb[k][:, j * NCHUNK : (j + 1) * NCHUNK],
                                        start=(k == 0),
                                        stop=(k == kt_used - 1),
                                    )
                            if do_evict:
                                d = dst[:, jj * 2 * NCHUNK : (jj + 1) * 2 * NCHUNK]
                                if (m + jj) % 2 == 0:
                                    nc.vector.tensor_scalar_add(d, ps[:], const_add)
                                else:
                                    nc.scalar.activation(
                                        d, ps[:],
                                        mybir.ActivationFunctionType.Copy,
                                        bias=const_add,
                                    )
                    base = g * sgroup * P
                    if not do_store:
                        pass
                    elif g < NG - 1:
                        eng = nc.sync if g % 2 == 0 else nc.scalar
                        eng.dma_start(
                            c[base : base + sgroup * P, :].rearrange(
                                "(h p) n -> p h n", p=P),
                            ot[:],
                        )
                    else:
                        # tail split: all-but-2 m-tiles in one piece,
                        # then 1 m-tile, then halves of the last - the
                        # serial drain after the final eviction is one
                        # 0.25MB piece per ring.
                        g2 = sgroup - 2
                        if g2:
                            nc.sync.dma_start(
                                c[base : base + g2 * P, :].rearrange(
                                    "(h p) n -> p h n", p=P),
                                ot[:, 0 : g2 * NS],
                            )
                        nc.scalar.dma_start(
                            c[base + g2 * P : base + (g2 + 1) * P, :],
                            ot[:, g2 * NS : (g2 + 1) * NS],
                        )
                        nc.sync.dma_start(
                            c[base + (g2 + 1) * P : base + sgroup * P, 0:half],
                            ot[:, (g2 + 1) * NS : (g2 + 1) * NS + half],
                        )
                        nc.scalar.dma_start(
                            c[base + (g2 + 1) * P : base + sgroup * P, half:NS],
                            ot[:, (g2 + 1) * NS + half : sgroup * NS],
                        )

            if repeat == 1:
                emit_loads(at_sets[0], b_sets[0])
                emit_compute(at_sets[0], b_sets[0], warm=True)
            elif not unroll2:
                with tc.For_i(0, repeat, 1, **(loop_opts or {})):
                    emit_loads(at_sets[0], b_sets[0])
                    emit_compute(at_sets[0], b_sets[0], warm=True)
            else:
                # prologue: load+run set 0, prefetch set 1 behind it
                emit_loads(at_sets[0], b_sets[0])
                emit_loads(at_sets[1], b_sets[1], fine=False)
                emit_compute(at_sets[0], b_sets[0], warm=True)
                n2, rem = divmod(repeat - 1, 2)
                if n2:
                    with tc.For_i(0, n2, 1, **(loop_opts or {})):
                        # each body prefetches the other set's inputs
                        emit_loads(at_sets[0], b_sets[0], fine=False)
                        emit_compute(at_sets[1], b_sets[1], warm=False)
                        emit_loads(at_sets[1], b_sets[1], fine=False)
                        emit_compute(at_sets[0], b_sets[0], warm=False)
                if rem:
                    emit_compute(at_sets[1], b_sets[1], warm=False)

    nc.compile()
    if ldw_dedup:
        _dedup_ldweights(nc)
    return nc


def _dedup_ldweights(nc):
    """Post-compile: drop InstLdweights whose weights AP equals the
    currently-loaded one. bacc splits every matmul into an explicit
    InstLdweights + InstMatmult(ldweights=False) pair, so with a
    same-weight-run matmul order most loads are redundant. Only loads
    with no waits/updates are dropped; tracking resets at block
    boundaries and drains."""
    f = nc.m.functions[0]
    for bb in f.blocks:
        insts = bb.instructions
        out = []
        loaded = None
        for inst in insts:
            nm = type(inst).__name__
            if str(getattr(inst, "engine", "")) != "EngineType.PE":
                out.append(inst)
                continue
            if nm == "InstLdweights":
                sig = str(inst.ins[0])
                if (sig == loaded and not getattr(inst, "waits", None)
                        and not getattr(inst, "updates", None)):
                    continue  # redundant reload
                loaded = sig
                out.append(inst)
            elif nm == "InstMatmult":
                out.append(inst)
            else:
                loaded = None  # drain / branch / unknown: be conservative
                out.append(inst)
        if len(out) != len(insts):
            bb.instructions = out


_CACHE = {}


def _get_program(const_add: float):
    key = const_add
    if key not in _CACHE:
        _CACHE[key] = build_program(const_add)
    return _CACHE[key]


def make_in_maps(A, B, in_dtype: str = "fp8"):
    """2x4 (M, N) grid; shards host-cast to fp8/bf16, A staged K-major."""
    np_dt = F8_NP if in_dtype == "fp8" else BF16_NP
    A16 = np.asarray(A, dtype=np.float32).astype(np_dt)
    B16 = np.asarray(B, dtype=np.float32).astype(np_dt)
    maps = []
    for i in range(NCORES):
        mi, ni = divmod(i, RN)
        maps.append({
            "at": np.ascontiguousarray(A16[mi * MS : (mi + 1) * MS].T),
            "b": np.ascontiguousarray(B16[:, ni * NS : (ni + 1) * NS]),
        })
    return maps


def assemble(results):
    rows = []
    for mi in range(RM):
        rows.append(np.concatenate(
            [results[mi * RN + ni]["c"] for ni in range(RN)], axis=1))
    return np.concatenate(rows, axis=0).astype(np.float32)


def run(A, B, world_size, trace=False, **spmd_kwargs):
    ws = int(world_size)
    const_add = float(ws * (ws + 1) / 2)
    A = np.asarray(A)
    B = np.asarray(B)
    assert A.shape == (M, K) and B.shape == (K, N)

    nc = _get_program(const_add)
    res = run_bass_kernel_spmd(
        nc, make_in_maps(A, B), list(range(NCORES)), trace=trace, **spmd_kwargs
    )
    return assemble(res.results), res


def kernel(A, B, world_size, **_unused):
    out, _ = run(A, B, world_size, trace=False)
    return out
